# revision 48
# baseline (speedup 1.0000x reference)
"""Trainium2 Bass kernel for nn_AtenMmQuint8: quint8 dense matmul.

    out = ((x - 65) * 0.199) @ ((y - 160) * 0.0215)
    x: [2048, 4096] int32 (quint8 values 0..255)
    y: [4096, 2048] int32 (quint8 values 0..255)
    out: [2048, 2048] fp32

Sharding: 4x2 tensor-parallel grid over the 8 NeuronCores (4 M-blocks x
2 N-blocks); per-core DMA traffic is minimized at this grid shape and
each core's matmul work is identical (256 PE matmuls at the 216ns
N=512 bf16 issue rate -> 55.25us PE floor; measured kernels sit within
~1.5us of it, the remainder being a fixed ~432ns/10.8us engine tick and
start-phase clock ramp).

Host staging: the zero-point subtraction is done on the host for free:
(x - 65) in [-65, 190] and (y - 160) in [-160, 95] are integers, exactly
representable in bf16 (integers up to 256 are exact), so we ship bf16
operands and the device does NO dequant casts at all. This doubles DMA
bytes (12MB/core load, ~223 GB/s average demand vs ~290 GB/s measured
aggregate) but frees the Vector engine entirely during the matmul
stream and removes every cast-wait stall from the PE critical path.
x is staged K-major (transposed) so the PE's stationary operand loads
directly. (Sub-55us alternatives were checked and are dead ends: the
cayman ISA removed UINT8 matmul, and fp8-DoubleRow exact hi/lo
splitting needs 2x the MACs for at most ~1.8x the rate.)

Device kernel (identical SPMD program on all 8 cores):
  - K is interleaved across SBUF partitions (k = p*kt + j) so each
    load-chunk DMA is 128 large contiguous runs (one per partition);
    the contraction is a permutation of K applied identically to both
    operands, so the matmul result is unchanged.
  - Both HWDGE rings (SP + ACT) carry interleaved x/y pieces ordered
    exactly by the PE's consumption order: per-k-tile alternation
    early (the supply margin over the PE's 1.72us/k-tile demand is
    only ~25%, so ordering is everything), tapering to 2- and 4-tile
    chunks as slack accumulates. The first two y tiles ship as column
    halves and k-tiles 0/1 run their matmuls n-outer, so the first
    matmuls gate on 128KB transfers.
  - PE prewarm: throwaway matmuls from right after the framework entry
    barrier keep the PE CONTINUOUSLY busy until the first data lands --
    any idle gap resets the HAM activity window and restarts the
    1.2GHz->2.4GHz ramp (a ~3.4us penalty if it bites mid-stream).
  - PE matmul bf16 x bf16 -> fp32, accumulating the whole 512x1024
    block across all 8 PSUM banks k-outer (PE never waits on a full K
    pass); the last 8 k-tiles run (m, n)-major so banks retire one at
    a time and their copy+store overlaps the remaining matmuls,
    keeping the kernel-ending chain short (one 512-col scale-copy +
    one 256KB store).
  - Scale+copy PSUM -> SBUF fused with the combined scale on VectorE
    (otherwise idle), one store DMA per retired PSUM bank on the SP
    ring (a second ring pays a cold-start latency penalty and loses).
"""

import numpy as np

import concourse.bass as bass  # noqa: F401  (kept for callers/debugging)
import concourse.mybir as mybir
import concourse.tile as tile
from concourse import bacc
from concourse.bass_utils import run_bass_kernel_spmd

X_ZP, Y_ZP = 65.0, 160.0
SCALE = 0.199 * 0.0215

M, K, N = 2048, 4096, 2048
GM, GN = 4, 2  # core grid: 4 M-blocks x 2 N-blocks
MC, NC = M // GM, N // GN  # 512 x 1024 per-core output block
P = 128  # partitions / k-tile size
NB = 512  # psum bank free size (one fp32 bank; matmul cannot cross banks)
# k-tiles per load DMA chunk: small leading chunks start the pipeline
# early (the PE only ever waits on a 1-2 k-tile transfer), moderate
# trailing chunks amortize per-DMA completion overhead while keeping
# the wait granularity fine enough that one chunk's completion latency
# (~1-2us receipt) never outruns the PE's 1.72us/k-tile consumption.
# Load schedules: one FIFO list per HWDGE ring, interleaving both
# tensors so each ring's delivery tracks the PE's consumption order.
# y needs 2/3 of the early bandwidth (256KB vs 128KB per k-tile), so
# the early odd y tiles ride the sync ring between x singles while the
# scalar ring carries the even ones; both rings then taper to bulk
# chunks once the pipeline has slack. ('y', a, b) = y tiles [a, b).
# ('yh', j, 0/1) = column half of y k-tile j -- the first two y tiles
# go in halves so the first matmuls of j=0/j=1 (n-outer order) each
# wait on a 128KB transfer instead of 256KB.
SYNC_SCHED = (
    ("x", 0, 1), ("yh", 0, 1), ("yh", 1, 0), ("x", 2, 3), ("y", 3, 4),
    ("x", 4, 5), ("y", 5, 6), ("x", 6, 7), ("y", 7, 8), ("x", 8, 10),
    ("y", 10, 12), ("x", 12, 14), ("y", 14, 16), ("x", 16, 18),
    ("x", 18, 20), ("y", 20, 22), ("y", 22, 24), ("x", 24, 28),
    ("y", 28, 32),
)
SCALAR_SCHED = (
    ("yh", 0, 0), ("x", 1, 2), ("yh", 1, 1), ("y", 2, 3), ("x", 3, 4),
    ("y", 4, 5), ("x", 5, 6), ("y", 6, 7), ("x", 7, 8), ("y", 8, 10),
    ("x", 10, 12), ("y", 12, 14), ("x", 14, 16), ("y", 16, 18),
    ("y", 18, 20), ("x", 20, 22), ("x", 22, 24), ("y", 24, 28),
    ("x", 28, 32),
)
KT_TAIL = 8  # trailing k-tiles run (m,n)-major so PSUM banks retire early
N_WARM = 30


def _emit(tc, xT, ys, out, sync_sched=SYNC_SCHED, scalar_sched=SCALAR_SCHED,
          kt_tail=KT_TAIL, n_warm=N_WARM):
    """Emit the per-core device program.

    xT: [k, mc] bf16 DRAM (x slice, K-major, zero-point subtracted),
    ys: [k, nnc] bf16 DRAM (zero-point subtracted),
    out: [mc, nnc] fp32 DRAM.
    """
    nc = tc.nc
    k, mc = xT.shape
    nnc = ys.shape[1]
    kt = k // P
    mt = mc // P
    nt = nnc // NB
    cover = {("x", j): 0 for j in range(kt)}
    cover.update({("y", j): 0.0 for j in range(kt)})
    for sched in (sync_sched, scalar_sched):
        for item in sched:
            if item[0] == "yh":
                cover[("y", item[1])] += 0.5
            elif item[0] in ("x", "y"):
                for j in range(item[1], item[2]):
                    cover[(item[0], j)] += 1
    assert all(v == 1 for v in cover.values()), cover

    fp32 = mybir.dt.float32
    bf16 = mybir.dt.bfloat16

    with (
        tc.tile_pool(name="sb", bufs=1) as sbp,
        tc.tile_pool(name="osb", bufs=mt * nt, space="SBUF") as osbp,
        tc.tile_pool(name="ps", bufs=mt * nt, space="PSUM") as psp,
    ):
        # Everything is persistent (fits in SBUF at this problem size):
        # each DMA writes a disjoint slice, so instructions don't accrue
        # buffer-recycling waits.
        xb = sbp.tile([P, kt, mc], bf16, name="xb")
        yb = sbp.tile([P, kt, nnc], bf16, name="yb")
        wt = sbp.tile([P, P], bf16, name="wt")
        psum = [
            [psp.tile([P, NB], fp32, tag="ps", name=f"ps_{m}_{n}") for n in range(nt)]
            for m in range(mt)
        ]

        # K interleaved across partitions (k = p*kt + j): each
        # partition's j-range is one contiguous DRAM run, so a chunk DMA
        # is 128 big descriptors instead of 128*nk small ones.
        xTr = xT.rearrange("(p j) m -> p j m", j=kt)
        ysr = ys.rearrange("(p j) n -> p j n", j=kt)
        # Each ring is FIFO: transfers happen in the issue order below,
        # at ~140-180 B/ns per ring while both are active.
        def issue(eng, sched):
            for item in sched:
                if item[0] == "yh":
                    j, h = item[1], item[2]
                    cs = slice(h * NB, (h + 1) * NB)
                    eng.dma_start(yb[:, j, cs], ysr[:, j, cs])
                elif item[0] == "x":
                    a, b = item[1], item[2]
                    eng.dma_start(xb[:, a:b, :], xTr[:, a:b, :])
                else:
                    a, b = item[1], item[2]
                    eng.dma_start(yb[:, a:b, :], ysr[:, a:b, :])

        issue(nc.sync, sync_sched)
        issue(nc.scalar, scalar_sched)

        # HAM prewarm: the PE sits idle for ~3 us while the first chunk
        # loads; throwaway matmuls release the clock gate to 8/8 before
        # the real stream starts. The PE must stay CONTINUOUSLY busy
        # from here through the real stream -- an idle gap resets the
        # HAM activity window and the first real matmuls run at 1.2GHz.
        # memset on VectorE (otherwise idle until the PSUM copies);
        # GpSimd placement was tried and lands at the same post-barrier
        # time, not the pre-barrier slot the framework's own memsets get.
        nc.vector.memset(wt[:], 0.0)
        for _ in range(n_warm):
            nc.tensor.matmul(psum[0][0][:, :P], wt[:], wt[:], start=True, stop=True)

        def mm(j, m, n):
            nc.tensor.matmul(
                psum[m][n][:],
                xb[:, j, m * P : (m + 1) * P],
                yb[:, j, n * NB : (n + 1) * NB],
                start=(j == 0),
                stop=(j == kt - 1),
            )

        # k-outer: touch every psum bank each k-tile so the PE stream
        # stays dense while loads race ahead. The first two k-tiles run
        # n-outer so their first 4 matmuls each need only the first y
        # column half (loaded by the smaller leading DMAs).
        #
        # Robustness carve-out: the last bank (mt-1, nt-1) skips k-slots
        # 0..7 and catches up with a second matmul per slot in slots
        # 8 and 12..18 (PSUM accumulation is k-order independent). The
        # 7-matmul early slots make the PE reach each k-tile's wait
        # sooner, so on a slow-DMA run the supply deficit is absorbed
        # as many small stalls instead of one multi-us lump -- a lumped
        # idle of ~3.4us resets the HAM clock ramp and costs ~4.5us
        # (measured both ways).
        last_b = (mt - 1, nt - 1)
        for j in (0, 1):
            for n in range(nt):
                for m in range(mt):
                    if (m, n) != last_b:
                        mm(j, m, n)
        for j in range(2, 8):
            for m in range(mt):
                for n in range(nt):
                    if (m, n) != last_b:
                        mm(j, m, n)
        for j in range(8, kt - kt_tail):
            for m in range(mt):
                for n in range(nt):
                    if (m, n) == last_b:
                        if j == 8:
                            # bank's first matmul: start=True on tile 0
                            mm(0, m, n)
                        elif 12 <= j < 19:
                            mm(j - 11, m, n)
                    mm(j, m, n)
        # (m, n)-major tail: each PSUM bank finishes its K accumulation
        # alone, so its scale-copy + store overlaps the remaining
        # matmuls of the other banks.
        for m in range(mt):
            for n in range(nt):
                for j in range(kt - kt_tail, kt):
                    mm(j, m, n)
                osb = osbp.tile([P, NB], fp32, tag="osb", name=f"osb_{m}_{n}")
                nc.vector.tensor_scalar_mul(osb[:], psum[m][n][:], SCALE)
                nc.sync.dma_start(
                    out[m * P : (m + 1) * P, n * NB : (n + 1) * NB], osb[:]
                )


def _build_nc(k=K, mc=MC, nnc=NC, **emit_kw):
    nc = bacc.Bacc("TRN2", target_bir_lowering=False, debug=False)
    xT = nc.declare_dram_parameter("xT", [k, mc], mybir.dt.bfloat16, isOutput=False)
    ys = nc.declare_dram_parameter("ys", [k, nnc], mybir.dt.bfloat16, isOutput=False)
    out = nc.declare_dram_parameter("out", [mc, nnc], mybir.dt.float32, isOutput=True)
    with tile.TileContext(nc) as tc:
        _emit(tc, xT[:], ys[:], out[:], **emit_kw)
    nc.compile()
    return nc


_CACHE = {}


def _get_nc():
    if "nc" not in _CACHE:
        _CACHE["nc"] = _build_nc()
    return _CACHE["nc"]


def kernel(x, y):
    x = np.asarray(x)
    y = np.asarray(y)
    assert x.shape == (M, K) and y.shape == (K, N)
    bf16 = mybir.dt.np(mybir.dt.bfloat16)
    # Zero-point subtraction on the host: the results are integers in
    # [-160, 190], exactly representable in bf16, so the device needs no
    # dequant work at all. x is staged K-major for the PE's stationary
    # operand.
    xT_bf = (x.T.astype(np.float32) - X_ZP).astype(bf16)
    y_bf = (y.astype(np.float32) - Y_ZP).astype(bf16)

    in_maps = []
    for i in range(GM * GN):
        mi, ni = divmod(i, GN)
        in_maps.append(
            {
                "xT": np.ascontiguousarray(xT_bf[:, mi * MC : (mi + 1) * MC]),
                "ys": np.ascontiguousarray(y_bf[:, ni * NC : (ni + 1) * NC]),
            }
        )

    res = run_bass_kernel_spmd(_get_nc(), in_maps, list(range(GM * GN)))
    _CACHE["last_results"] = res

    out = np.empty((M, N), np.float32)
    for i in range(GM * GN):
        mi, ni = divmod(i, GN)
        out[mi * MC : (mi + 1) * MC, ni * NC : (ni + 1) * NC] = res.results[i]["out"]
    return out


# revision 49
# speedup vs baseline: 1.0136x; 1.0136x over previous
"""Trainium2 Bass kernel for nn_AtenMmQuint8: quint8 dense matmul.

    out = ((x - 65) * 0.199) @ ((y - 160) * 0.0215)
    x: [2048, 4096] int32 (quint8 values 0..255)
    y: [4096, 2048] int32 (quint8 values 0..255)
    out: [2048, 2048] fp32

Sharding: 4x2 tensor-parallel grid over the 8 NeuronCores (4 M-blocks x
2 N-blocks); per-core DMA traffic is minimized at this grid shape and
each core's matmul work is identical (256 PE matmuls at the 216ns
N=512 bf16 issue rate -> 55.25us PE floor; measured kernels sit within
~1.5us of it, the remainder being a fixed ~432ns/10.8us engine tick and
start-phase clock ramp).

Host staging: the zero-point subtraction is done on the host for free:
(x - 65) in [-65, 190] and (y - 160) in [-160, 95] are integers, exactly
representable in bf16 (integers up to 256 are exact), so we ship bf16
operands and the device does NO dequant casts at all. This doubles DMA
bytes (12MB/core load, ~223 GB/s average demand vs ~290 GB/s measured
aggregate) but frees the Vector engine entirely during the matmul
stream and removes every cast-wait stall from the PE critical path.
x is staged K-major (transposed) so the PE's stationary operand loads
directly. (Sub-55us alternatives were checked and are dead ends: the
cayman ISA removed UINT8 matmul, and fp8-DoubleRow exact hi/lo
splitting needs 2x the MACs for at most ~1.8x the rate.)

Device kernel (identical SPMD program on all 8 cores):
  - K is interleaved across SBUF partitions (k = p*kt + j) so each
    load-chunk DMA is 128 large contiguous runs (one per partition);
    the contraction is a permutation of K applied identically to both
    operands, so the matmul result is unchanged.
  - Both HWDGE rings (SP + ACT) carry interleaved x/y pieces ordered
    exactly by the PE's consumption order: per-k-tile alternation
    early (the supply margin over the PE's 1.72us/k-tile demand is
    only ~25%, so ordering is everything), tapering to 2- and 4-tile
    chunks as slack accumulates. The first two y tiles ship as column
    halves and k-tiles 0/1 run their matmuls n-outer, so the first
    matmuls gate on 128KB transfers.
  - PE prewarm: throwaway matmuls from right after the framework entry
    barrier keep the PE CONTINUOUSLY busy until the first data lands --
    any idle gap resets the HAM activity window and restarts the
    1.2GHz->2.4GHz ramp (a ~3.4us penalty if it bites mid-stream).
  - PE matmul bf16 x bf16 -> fp32, accumulating the whole 512x1024
    block across all 8 PSUM banks k-outer (PE never waits on a full K
    pass); the last 8 k-tiles run (m, n)-major so banks retire one at
    a time and their copy+store overlaps the remaining matmuls,
    keeping the kernel-ending chain short (one 512-col scale-copy +
    one 256KB store).
  - Scale+copy PSUM -> SBUF fused with the combined scale on VectorE
    (otherwise idle), one store DMA per retired PSUM bank on the SP
    ring (a second ring pays a cold-start latency penalty and loses).
"""

import numpy as np

import concourse.bass as bass  # noqa: F401  (kept for callers/debugging)
import concourse.mybir as mybir
import concourse.tile as tile
from concourse import bacc
from concourse.bass_utils import run_bass_kernel_spmd

X_ZP, Y_ZP = 65.0, 160.0
SCALE = 0.199 * 0.0215

M, K, N = 2048, 4096, 2048
GM, GN = 4, 2  # core grid: 4 M-blocks x 2 N-blocks
MC, NC = M // GM, N // GN  # 512 x 1024 per-core output block
P = 128  # partitions / k-tile size
NB = 512  # psum bank free size (one fp32 bank; matmul cannot cross banks)
# k-tiles per load DMA chunk: small leading chunks start the pipeline
# early (the PE only ever waits on a 1-2 k-tile transfer), moderate
# trailing chunks amortize per-DMA completion overhead while keeping
# the wait granularity fine enough that one chunk's completion latency
# (~1-2us receipt) never outruns the PE's 1.72us/k-tile consumption.
# Load schedules: one FIFO list per HWDGE ring, interleaving both
# tensors so each ring's delivery tracks the PE's consumption order.
# y needs 2/3 of the early bandwidth (256KB vs 128KB per k-tile), so
# the early odd y tiles ride the sync ring between x singles while the
# scalar ring carries the even ones; both rings then taper to bulk
# chunks once the pipeline has slack. ('y', a, b) = y tiles [a, b).
# ('yh', j, 0/1) = column half of y k-tile j -- the first two y tiles
# go in halves so the first matmuls of j=0/j=1 (n-outer order) each
# wait on a 128KB transfer instead of 256KB.
SYNC_SCHED = (
    ("yh", 0, 0), ("yh", 0, 1), ("yh", 1, 0), ("x", 2, 3), ("y", 3, 4),
    ("x", 4, 5), ("y", 5, 6), ("x", 6, 7), ("y", 7, 8), ("x", 8, 10),
    ("y", 10, 12), ("x", 12, 14), ("y", 14, 16), ("x", 16, 18),
    ("x", 18, 20), ("y", 20, 22), ("y", 22, 24), ("x", 24, 28),
    ("y", 28, 32),
)
SCALAR_SCHED = (
    ("x", 0, 1), ("x", 1, 2), ("yh", 1, 1), ("y", 2, 3), ("x", 3, 4),
    ("y", 4, 5), ("x", 5, 6), ("y", 6, 7), ("x", 7, 8), ("y", 8, 10),
    ("x", 10, 12), ("y", 12, 14), ("x", 14, 16), ("y", 16, 18),
    ("y", 18, 20), ("x", 20, 22), ("x", 22, 24), ("y", 24, 28),
    ("x", 28, 32),
)
KT_TAIL = 8  # trailing k-tiles run (m,n)-major so PSUM banks retire early
N_WARM = 30


def _emit(tc, xT, ys, out, sync_sched=SYNC_SCHED, scalar_sched=SCALAR_SCHED,
          kt_tail=KT_TAIL, n_warm=N_WARM):
    """Emit the per-core device program.

    xT: [k, mc] bf16 DRAM (x slice, K-major, zero-point subtracted),
    ys: [k, nnc] bf16 DRAM (zero-point subtracted),
    out: [mc, nnc] fp32 DRAM.
    """
    nc = tc.nc
    k, mc = xT.shape
    nnc = ys.shape[1]
    kt = k // P
    mt = mc // P
    nt = nnc // NB
    cover = {("x", j): 0 for j in range(kt)}
    cover.update({("y", j): 0.0 for j in range(kt)})
    for sched in (sync_sched, scalar_sched):
        for item in sched:
            if item[0] == "yh":
                cover[("y", item[1])] += 0.5
            elif item[0] in ("x", "y"):
                for j in range(item[1], item[2]):
                    cover[(item[0], j)] += 1
    assert all(v == 1 for v in cover.values()), cover

    fp32 = mybir.dt.float32
    bf16 = mybir.dt.bfloat16

    with (
        tc.tile_pool(name="sb", bufs=1) as sbp,
        tc.tile_pool(name="osb", bufs=mt * nt, space="SBUF") as osbp,
        tc.tile_pool(name="ps", bufs=mt * nt, space="PSUM") as psp,
    ):
        # Everything is persistent (fits in SBUF at this problem size):
        # each DMA writes a disjoint slice, so instructions don't accrue
        # buffer-recycling waits.
        xb = sbp.tile([P, kt, mc], bf16, name="xb")
        yb = sbp.tile([P, kt, nnc], bf16, name="yb")
        wt = sbp.tile([P, P], bf16, name="wt")
        psum = [
            [psp.tile([P, NB], fp32, tag="ps", name=f"ps_{m}_{n}") for n in range(nt)]
            for m in range(mt)
        ]

        # K interleaved across partitions (k = p*kt + j): each
        # partition's j-range is one contiguous DRAM run, so a chunk DMA
        # is 128 big descriptors instead of 128*nk small ones.
        xTr = xT.rearrange("(p j) m -> p j m", j=kt)
        ysr = ys.rearrange("(p j) n -> p j n", j=kt)
        # Each ring is FIFO: transfers happen in the issue order below,
        # at ~140-180 B/ns per ring while both are active.
        def issue(eng, sched):
            for item in sched:
                if item[0] == "yh":
                    j, h = item[1], item[2]
                    cs = slice(h * NB, (h + 1) * NB)
                    eng.dma_start(yb[:, j, cs], ysr[:, j, cs])
                elif item[0] == "x":
                    a, b = item[1], item[2]
                    eng.dma_start(xb[:, a:b, :], xTr[:, a:b, :])
                else:
                    a, b = item[1], item[2]
                    eng.dma_start(yb[:, a:b, :], ysr[:, a:b, :])

        issue(nc.sync, sync_sched)
        issue(nc.scalar, scalar_sched)

        # HAM prewarm: the PE sits idle for ~3 us while the first chunk
        # loads; throwaway matmuls release the clock gate to 8/8 before
        # the real stream starts. The PE must stay CONTINUOUSLY busy
        # from here through the real stream -- an idle gap resets the
        # HAM activity window and the first real matmuls run at 1.2GHz.
        # memset on VectorE (otherwise idle until the PSUM copies);
        # GpSimd placement was tried and lands at the same post-barrier
        # time, not the pre-barrier slot the framework's own memsets get.
        nc.vector.memset(wt[:], 0.0)
        for _ in range(n_warm):
            nc.tensor.matmul(psum[0][0][:, :P], wt[:], wt[:], start=True, stop=True)

        def mm(j, m, n):
            nc.tensor.matmul(
                psum[m][n][:],
                xb[:, j, m * P : (m + 1) * P],
                yb[:, j, n * NB : (n + 1) * NB],
                start=(j == 0),
                stop=(j == kt - 1),
            )

        # k-outer: touch every psum bank each k-tile so the PE stream
        # stays dense while loads race ahead. The first two k-tiles run
        # n-outer so their first 4 matmuls each need only the first y
        # column half (loaded by the smaller leading DMAs).
        #
        # Robustness carve-out: the last bank (mt-1, nt-1) skips k-slots
        # 0..7 and catches up with a second matmul per slot in slots
        # 8 and 12..18 (PSUM accumulation is k-order independent). The
        # 7-matmul early slots make the PE reach each k-tile's wait
        # sooner, so on a slow-DMA run the supply deficit is absorbed
        # as many small stalls instead of one multi-us lump -- a lumped
        # idle of ~3.4us resets the HAM clock ramp and costs ~4.5us
        # (measured both ways).
        last_b = (mt - 1, nt - 1)
        for j in (0, 1):
            for n in range(nt):
                for m in range(mt):
                    if (m, n) != last_b:
                        mm(j, m, n)
        for j in range(2, 8):
            for m in range(mt):
                for n in range(nt):
                    if (m, n) != last_b:
                        mm(j, m, n)
        for j in range(8, kt - kt_tail):
            for m in range(mt):
                for n in range(nt):
                    if (m, n) == last_b:
                        if j == 8:
                            # bank's first matmul: start=True on tile 0
                            mm(0, m, n)
                        elif 12 <= j < 19:
                            mm(j - 11, m, n)
                    mm(j, m, n)
        # (m, n)-major tail: each PSUM bank finishes its K accumulation
        # alone, so its scale-copy + store overlaps the remaining
        # matmuls of the other banks.
        for m in range(mt):
            for n in range(nt):
                for j in range(kt - kt_tail, kt):
                    mm(j, m, n)
                osb = osbp.tile([P, NB], fp32, tag="osb", name=f"osb_{m}_{n}")
                nc.vector.tensor_scalar_mul(osb[:], psum[m][n][:], SCALE)
                nc.sync.dma_start(
                    out[m * P : (m + 1) * P, n * NB : (n + 1) * NB], osb[:]
                )


def _build_nc(k=K, mc=MC, nnc=NC, **emit_kw):
    nc = bacc.Bacc("TRN2", target_bir_lowering=False, debug=False)
    xT = nc.declare_dram_parameter("xT", [k, mc], mybir.dt.bfloat16, isOutput=False)
    ys = nc.declare_dram_parameter("ys", [k, nnc], mybir.dt.bfloat16, isOutput=False)
    out = nc.declare_dram_parameter("out", [mc, nnc], mybir.dt.float32, isOutput=True)
    with tile.TileContext(nc) as tc:
        _emit(tc, xT[:], ys[:], out[:], **emit_kw)
    nc.compile()
    return nc


_CACHE = {}


def _get_nc():
    if "nc" not in _CACHE:
        _CACHE["nc"] = _build_nc()
    return _CACHE["nc"]


def kernel(x, y):
    x = np.asarray(x)
    y = np.asarray(y)
    assert x.shape == (M, K) and y.shape == (K, N)
    bf16 = mybir.dt.np(mybir.dt.bfloat16)
    # Zero-point subtraction on the host: the results are integers in
    # [-160, 190], exactly representable in bf16, so the device needs no
    # dequant work at all. x is staged K-major for the PE's stationary
    # operand.
    xT_bf = (x.T.astype(np.float32) - X_ZP).astype(bf16)
    y_bf = (y.astype(np.float32) - Y_ZP).astype(bf16)

    in_maps = []
    for i in range(GM * GN):
        mi, ni = divmod(i, GN)
        in_maps.append(
            {
                "xT": np.ascontiguousarray(xT_bf[:, mi * MC : (mi + 1) * MC]),
                "ys": np.ascontiguousarray(y_bf[:, ni * NC : (ni + 1) * NC]),
            }
        )

    res = run_bass_kernel_spmd(_get_nc(), in_maps, list(range(GM * GN)))
    _CACHE["last_results"] = res

    out = np.empty((M, N), np.float32)
    for i in range(GM * GN):
        mi, ni = divmod(i, GN)
        out[mi * MC : (mi + 1) * MC, ni * NC : (ni + 1) * NC] = res.results[i]["out"]
    return out
